# revision 31
# baseline (speedup 1.0000x reference)
"""Trainium2 Bass kernel for a dense transformer layer (attention + FFN + 2 LayerNorms).

Problem shapes: x [4, 2048, 1024], d_model=1024, heads=16 (hd=64), d_ff=4096.

Sharding: 8 cores; core c handles batch b = c//2, sequence half = c%2
(1024 query tokens).  Each core computes K/V for its batch's full 2048
tokens (duplicated across the pair - avoids cross-core communication).
The host permutes each core's x so its own token half comes first.

v2 layout/pipeline strategy:
 - Activations feature-major (d_model on partitions) so weights serve as
   matmul stationaries.  V is projected token-major directly (x-tile
   stationary, Wv moving) - no PE transposes.
 - Scores S^T = K @ Q^T per head with a 64-row contraction: the two heads
   of a feature-tile pair are issued as row-tiled matmuls (rows 0-63 /
   64-127) into different PSUM banks, so they run concurrently on the PE
   array (2x scores throughput) and need no zero-padded staging.
 - Softmax exp is the scalar-engine bottleneck (~275us); all other
   evictions run on the vector engine.  The query dimension is split in
   two 512-token chunks and the whole network is software-pipelined so
   the PE keeps busy during the exp-bound attention windows:
     P1: V proj, Q proj(chunk0), K proj(head-pair 0)
     P2: attention(chunk0) with K proj pairs 1-7 + Q proj(chunk1) fillers
     P3: Wo + LN1 (chunk0)
     P4: attention(chunk1) with the whole FFN(chunk0) interleaved
     P5: Wo + LN1 (chunk1), FFN(chunk1) with LN2+output(chunk0) interleaved
     P6: LN2 + output (chunk1)
 - Per-head-pair softmax denominators are evicted with the ctx rows via
   SBUF->SBUF DMA and each pair is normalized as soon as it finishes.
 - SBUF/PSUM tile pools are stack-scoped; lifetimes are nested so each
   phase's transient pools pop before longer-lived ones.
"""

import os
from collections import deque
from contextlib import ExitStack

import numpy as np

import concourse.bass as bass
import concourse.tile as tile
from concourse import bacc, mybir
from concourse import bass_utils

BF16 = mybir.dt.bfloat16
F32 = mybir.dt.float32
AF = mybir.ActivationFunctionType
OP = mybir.AluOpType

D = 1024          # d_model
S = 2048          # full sequence per batch
T = 1024          # query tokens per core
H = 16            # heads
HD = 64           # head dim
F = 4096          # ffn hidden
P = 128
DT = D // P       # 8 feature tiles
KT = S // P       # 16 key-token tiles
FT = F // P       # 32 hidden tiles
N_CORES = 8
EPS = 1e-5

_CACHED = {}


def _build_program():
    nc = bacc.Bacc("TRN2", target_bir_lowering=False, debug=False,
                   num_devices=N_CORES)

    tens = {}

    def di(name, shape, dtype=BF16):
        tens[name] = nc.dram_tensor(name, shape, dtype, kind="ExternalInput")

    di("xT", [D, S])
    di("wq", [D, D]); di("wk", [D, D]); di("wv", [D, D]); di("wo", [D, D])
    di("w1", [D, F]); di("w2", [F, D])
    for nm in ["bq_p", "bk_p", "bo_p", "b2_p", "g1_p", "be1_p"]:
        di(nm, [P, DT], F32)
    di("b1_p", [P, FT], F32)
    di("bv_r", [1, D])
    di("g2_d", [D]); di("be2_d", [D])
    di("ident_d", [P, P]); di("ones_row_d", [1, P]); di("ones_col_d", [P, 1])
    di("sel2_d", [2, P])
    tens["out"] = nc.dram_tensor("out", [T, D], F32, kind="ExternalOutput")

    with tile.TileContext(nc) as tc:
        _trace_kernel(nc, tc, tens)
    nc.compile()
    return nc


def _trace_kernel(nc, tc, t):
    xT, wq, wk, wv, wo, w1, w2 = (t["xT"], t["wq"], t["wk"], t["wv"], t["wo"],
                                  t["w1"], t["w2"])
    out = t["out"]

    es = ExitStack()
    with es:
        dram = es.enter_context(tc.tile_pool(name="dram", bufs=1, space="DRAM"))
        kT_hbm = dram.tile([D, S], BF16, tag="kh", name="kh")

        # ---------------- constants (bottom of SBUF stack) --------------
        # startup DMAs are split across the sync and scalar queues (the
        # scalar engine is idle until the first softmax) so descriptor
        # processing isn't serialized on one engine.
        const = es.enter_context(tc.tile_pool(name="const", bufs=1))
        ident = const.tile([P, P], BF16, tag="ident", name="ident")
        nc.scalar.dma_start(out=ident, in_=t["ident_d"][:, :])
        ones_row = const.tile([1, P], BF16, tag="onesr", name="onesr")
        nc.scalar.dma_start(out=ones_row, in_=t["ones_row_d"][:, :])
        ones_col = const.tile([P, 1], BF16, tag="onesc", name="onesc")
        nc.scalar.dma_start(out=ones_col, in_=t["ones_col_d"][:, :])
        sel2 = const.tile([2, P], BF16, tag="sel2", name="sel2")
        nc.scalar.dma_start(out=sel2, in_=t["sel2_d"][:, :])
        biases = {}
        for name in ["bq_p", "bk_p", "bo_p", "b2_p", "g1_p", "be1_p"]:
            bt = const.tile([P, DT], F32, tag=name)
            nc.scalar.dma_start(out=bt, in_=t[name][:, :])
            biases[name] = bt
        eps_sb = const.tile([P, 1], F32, tag="eps", name="eps")
        nc.vector.memset(eps_sb[:], EPS)
        b1_sb = const.tile([P, FT], F32, tag="b1", name="b1")
        nc.scalar.dma_start(out=b1_sb, in_=t["b1_p"][:, :])
        g2_bc = const.tile([P, D], BF16, tag="g2bc", name="g2bc")
        nc.scalar.dma_start(out=g2_bc, in_=bass.AP(
            tensor=t["g2_d"], offset=0, ap=[[0, P], [1, D]]))
        be2_bc = const.tile([P, D], BF16, tag="be2bc", name="be2bc")
        nc.scalar.dma_start(out=be2_bc, in_=bass.AP(
            tensor=t["be2_d"], offset=0, ap=[[0, P], [1, D]]))

        # ---------------- long-lived pools (es scope) -------------------
        xo_pool = es.enter_context(tc.tile_pool(name="xown", bufs=1))
        x_own = [xo_pool.tile([P, T], BF16, tag=f"xo{i}", name=f"xo{i}")
                 for i in range(DT)]
        hT_pool = es.enter_context(tc.tile_pool(name="hT", bufs=1))
        hT = [hT_pool.tile([P, T], BF16, tag=f"hT{i}", name=f"hT{i}")
              for i in range(DT)]
        ev_pool = es.enter_context(tc.tile_pool(name="cev", bufs=1))
        nrm_pool = es.enter_context(tc.tile_pool(name="nrm", bufs=1))
        pb_pool = es.enter_context(tc.tile_pool(name="pb", bufs=2))
        o2_pool = es.enter_context(tc.tile_pool(name="o2", bufs=1))
        out2 = {}

        # PSUM: pool2 (es, bottom) then pool1/pool3 (popped at P5)
        pool2 = es.enter_context(
            tc.tile_pool(name="pool2", bufs=2, space="PSUM"))

        # ---------------- mid-lived stack: ctx, kbuf, v, qt1 ------------
        ctx_cm = tc.tile_pool(name="ctxs", bufs=1)
        ctx_pool = ctx_cm.__enter__()
        ctxs = [ctx_pool.tile([P, T], BF16, tag=f"ctx{i}", name=f"ctx{i}")
                for i in range(DT)]
        kb_cm = tc.tile_pool(name="kbuf", bufs=2)
        kbuf_pool = kb_cm.__enter__()
        v_cm = tc.tile_pool(name="vsb", bufs=1)
        v_pool = v_cm.__enter__()
        v_sb = [v_pool.tile([P, H, HD + 1], BF16, tag=f"v{i}", name=f"v{i}")
                for i in range(KT)]
        qt1_cm = tc.tile_pool(name="qt1", bufs=1)
        qt1_pool = qt1_cm.__enter__()
        QTc = {1: [qt1_pool.tile([P, 512], BF16, tag=f"q{i}", name=f"q{i}")
                   for i in range(DT)]}

        pool1_cm = tc.tile_pool(name="pool1", bufs=1, space="PSUM")
        pool1 = pool1_cm.__enter__()
        pool3_cm = tc.tile_pool(name="pool3", bufs=1, space="PSUM")
        pool3 = pool3_cm.__enter__()

        # ---------------- P1/P2-era stack: wearly, xoth, wv -------------
        we_cm = tc.tile_pool(name="wearly", bufs=1)
        we_pool = we_cm.__enter__()
        xoth_cm = tc.tile_pool(name="xoth", bufs=1)
        xoth_pool = xoth_cm.__enter__()
        x_oth = [xoth_pool.tile([P, T], BF16, tag=f"xt{i}", name=f"xt{i}")
                 for i in range(DT)]

        def xmov(din, ch):
            if ch < 2:
                return x_own[din][:, ch * 512:(ch + 1) * 512]
            return x_oth[din][:, (ch - 2) * 512:(ch - 1) * 512]

        def load_w(pool, wd, tagp, eng=None):
            eng = eng or nc.sync
            w_sb = [pool.tile([P, D], BF16, tag=f"{tagp}{i}",
                              name=f"{tagp}{i}") for i in range(DT)]
            for i in range(DT):
                eng.dma_start(out=w_sb[i], in_=wd[i * P:(i + 1) * P, :])
            return w_sb

        wv_cm = tc.tile_pool(name="wv", bufs=1)
        wv_pool = wv_cm.__enter__()
        wv_sb = [wv_pool.tile([P, D], BF16, tag=f"wv{i}", name=f"wv{i}")
                 for i in range(DT)]
        bv_r = wv_pool.tile([1, D], BF16, tag="bvr", name="bvr")
        # Critical-path loads first (half-tile granularity so V proj can
        # start after ~2MB): x_own + wv on the sync queue.
        for i in range(DT):
            nc.sync.dma_start(out=x_own[i][:, 0:512],
                              in_=xT[i * P:(i + 1) * P, 0:512])
            nc.sync.dma_start(out=wv_sb[i][:, 0:512],
                              in_=wv[i * P:(i + 1) * P, 0:512])
        for i in range(DT):
            nc.sync.dma_start(out=x_own[i][:, 512:T],
                              in_=xT[i * P:(i + 1) * P, 512:T])
            nc.sync.dma_start(out=wv_sb[i][:, 512:D],
                              in_=wv[i * P:(i + 1) * P, 512:D])
        nc.sync.dma_start(out=bv_r, in_=t["bv_r"][:, :])
        # Background loads on the gpsimd queue, gated behind x_own arrival
        # by a tiny queue-ordering DMA so they don't steal HBM bandwidth
        # from the critical path.
        nc.gpsimd.dma_start(out=kT_hbm[0:1, 0:8], in_=x_own[DT - 1][0:1, 0:8])
        for i in range(DT):
            nc.gpsimd.dma_start(out=x_oth[i], in_=xT[i * P:(i + 1) * P, T:S])
        wk_sb = load_w(we_pool, wk, "wk", nc.gpsimd)
        wq_sb = load_w(we_pool, wq, "wq", nc.gpsimd)

        # =============== P1: V proj (token-major) =======================
        bv_bc = const.tile([P, H, HD], BF16, tag="bvbc", name="bvbc")
        for fg in range(2):
            ps = pool2.tile([P, 8, HD], F32, tag="t2", name="t2")
            nc.tensor.matmul(ps[:], ones_row[:],
                             bv_r[:, fg * 512:(fg + 1) * 512],
                             start=True, stop=True)
            nc.vector.tensor_copy(bv_bc[:, fg * 8:(fg + 1) * 8, :], ps[:])

        def v_group(tt, fg):
            xstat = x_own if tt < 8 else x_oth
            tc128 = (tt % 8) * P
            ps = pool2.tile([P, 8, HD], F32, tag="t2", name="t2")
            for din in range(DT):
                nc.tensor.matmul(
                    ps[:], xstat[din][:, tc128:tc128 + P],
                    wv_sb[din][:, fg * 512:(fg + 1) * 512],
                    start=(din == 0), stop=(din == DT - 1))
            nc.vector.tensor_tensor(
                out=v_sb[tt][:, fg * 8:(fg + 1) * 8, 0:HD],
                in0=ps[:], in1=bv_bc[:, fg * 8:(fg + 1) * 8, :],
                op=OP.add)

        for tt in range(8):
            v_group(tt, 0)
        for tt in range(8):
            v_group(tt, 1)
        for tt in range(8, KT):
            v_group(tt, 0)
            v_group(tt, 1)
        for tt in range(KT):
            nc.vector.memset(v_sb[tt][:, :, HD:HD + 1], 1.0)

        wv_cm.__exit__(None, None, None)
        qt0_cm = tc.tile_pool(name="qt0", bufs=1)
        qt0_pool = qt0_cm.__enter__()
        QTc[0] = [qt0_pool.tile([P, 512], BF16, tag=f"q{i}", name=f"q{i}")
                  for i in range(DT)]

        # =============== projection group emitters ======================
        def k_group(dout, ch, kt_dest):
            # K projection straight into the SBUF tile attention will read;
            # a gpsimd-queued DMA mirrors it to HBM for the chunk-1 pass.
            ps = pool2.tile([P, 512], F32, tag="t2", name="t2")
            for din in range(DT):
                nc.tensor.matmul(
                    ps[:], wk_sb[din][:, dout * P:(dout + 1) * P],
                    xmov(din, ch), start=(din == 0), stop=(din == DT - 1))
            csl = slice(ch * 512, (ch + 1) * 512)
            nc.vector.tensor_scalar(
                out=kt_dest[:, csl], in0=ps[:],
                scalar1=biases["bk_p"][:, dout:dout + 1],
                scalar2=None, op0=OP.add)
            nc.gpsimd.dma_start(
                out=kT_hbm[dout * P:(dout + 1) * P, csl],
                in_=kt_dest[:, csl])

        def q_group(dout, qc):
            ps = pool2.tile([P, 512], F32, tag="t2", name="t2")
            for din in range(DT):
                nc.tensor.matmul(
                    ps[:], wq_sb[din][:, dout * P:(dout + 1) * P],
                    x_own[din][:, qc * 512:(qc + 1) * 512],
                    start=(din == 0), stop=(din == DT - 1))
            nc.vector.tensor_scalar(
                out=QTc[qc][dout][:], in0=ps[:],
                scalar1=biases["bq_p"][:, dout:dout + 1],
                scalar2=None, op0=OP.add)

        for dout in range(DT):
            q_group(dout, 0)
        ksb_cur = kbuf_pool.tile([P, S], BF16, tag="kb", name="kb")
        for ch in range(4):
            k_group(0, ch, ksb_cur)

        # =============== attention machinery ============================
        def load_ksb(dt_):
            ksb = kbuf_pool.tile([P, S], BF16, tag="kb", name="kb")
            nc.gpsimd.dma_start(out=ksb, in_=kT_hbm[dt_ * P:(dt_ + 1) * P, :])
            return ksb

        def normalize(dt_, qc, den_pair):
            qsl = slice(qc * 512, (qc + 1) * 512)
            rcpf = nrm_pool.tile([2, 512], F32, tag="rcpf", name="rcpf")
            nc.vector.reciprocal(rcpf[:], den_pair[:])
            rcpb = nrm_pool.tile([2, 512], BF16, tag="rcpb", name="rcpb")
            nc.vector.tensor_copy(rcpb[:], rcpf[:])
            bcp = pool2.tile([P, 512], F32, tag="t2", name="t2")
            nc.tensor.matmul(bcp[:], sel2[:], rcpb[:], start=True, stop=True)
            nc.vector.tensor_tensor(out=ctxs[dt_][:, qsl],
                                    in0=ctxs[dt_][:, qsl], in1=bcp[:],
                                    op=OP.mult)

        def attn_pair(dt_, qc, ksb, fillers):
            qsl = slice(qc * 512, (qc + 1) * 512)
            sps = [pool1.tile([P, 2, 512], F32, tag=f"sps{hh}", name="sps")
                   for hh in range(2)]
            cps = [pool3.tile([P, 512], F32, tag=f"cps{hh}", name="cps")
                   for hh in range(2)]
            for jp in range(KT // 2):
                j0 = 2 * jp
                for jj in range(2):
                    j = j0 + jj
                    for hh in range(2):
                        r0 = hh * HD
                        nc.tensor.matmul(
                            sps[hh][:, jj, :],
                            ksb[r0:r0 + HD, j * P:(j + 1) * P],
                            QTc[qc][dt_][r0:r0 + HD, :],
                            start=True, stop=True)
                pT = [None, None]
                for hh in range(2):
                    pT[hh] = pb_pool.tile([P, 2, 512], BF16, tag="pT",
                                          name="pT")
                    nc.scalar.activation(pT[hh][:], sps[hh][:], AF.Exp)
                for jj in range(2):
                    j = j0 + jj
                    for hh in range(2):
                        h = 2 * dt_ + hh
                        nc.tensor.matmul(
                            cps[hh][0:HD + 1, :], v_sb[j][:, h, :],
                            pT[hh][:, jj, :],
                            start=(jp == 0 and jj == 0),
                            stop=(jp == KT // 2 - 1 and jj == 1))
                if fillers:
                    fillers.popleft()()
                if fillers:
                    fillers.popleft()()
            den_pair = nrm_pool.tile([2, 512], BF16, tag="den", name="den",
                                     bufs=2)
            for hh in range(2):
                stage = ev_pool.tile([HD + 1, 512], BF16, tag="ctxe",
                                     name="ctxe", bufs=2)
                nc.vector.tensor_copy(stage[:], cps[hh][0:HD + 1, :])
                nc.gpsimd.dma_start(
                    out=ctxs[dt_][hh * HD:(hh + 1) * HD, qsl],
                    in_=stage[0:HD, :])
                nc.gpsimd.dma_start(out=den_pair[hh:hh + 1, :],
                                    in_=stage[HD:HD + 1, :])
            return den_pair

        # =============== P2: attention chunk 0 ==========================
        dens0 = {}
        for p in range(DT):
            fillers = deque()
            if p < DT - 1:
                ksb_next = kbuf_pool.tile([P, S], BF16, tag="kb", name="kb")
                for ch in range(4):
                    fillers.append(
                        (lambda d=p + 1, c=ch, kt=ksb_next: k_group(d, c, kt)))
            else:
                ksb_next = None
                for dout in range(DT):
                    fillers.append((lambda d=dout: q_group(d, 1)))
            if p - 1 in dens0:
                fillers.append(
                    (lambda d=p - 1: normalize(d, 0, dens0.pop(d))))
            dens0[p] = attn_pair(p, 0, ksb_cur, fillers)
            while fillers:
                fillers.popleft()()
            ksb_cur = ksb_next

        qt0_cm.__exit__(None, None, None)
        xoth_cm.__exit__(None, None, None)
        we_cm.__exit__(None, None, None)

        # =============== Wo + LN1 units (per chunk) =====================
        ln1_cm = tc.tile_pool(name="ln1", bufs=1)
        ln1 = ln1_cm.__enter__()
        wo_cm = tc.tile_pool(name="wop", bufs=1)
        wo_pool = wo_cm.__enter__()
        wo_sb = load_w(wo_pool, wo, "wo", nc.gpsimd)

        def wo_ln1_units(qc, dens, zT_pool):
            qsl = slice(qc * 512, (qc + 1) * 512)
            zT = [zT_pool.tile([P, 512], BF16, tag=f"z{i}", name=f"z{i}")
                  for i in range(DT)]
            units = []

            def norm_rest():
                for d in sorted(dens):
                    normalize(d, qc, dens.pop(d))
            units.append(norm_rest)

            def wo_group(dout):
                ps = pool2.tile([P, 512], F32, tag="t2", name="t2")
                for din in range(DT):
                    nc.tensor.matmul(
                        ps[:], wo_sb[din][:, dout * P:(dout + 1) * P],
                        ctxs[din][:, qsl], start=(din == 0),
                        stop=(din == DT - 1))
                nc.vector.scalar_tensor_tensor(
                    zT[dout][:], ps[:], biases["bo_p"][:, dout:dout + 1],
                    x_own[dout][:, qsl], op0=OP.add, op1=OP.add)
            for dout in range(DT):
                units.append((lambda d=dout: wo_group(d)))

            def stats1():
                zsqs = []
                for dt_ in range(DT):
                    zsq = pb_pool.tile([P, 2, 512], BF16, tag="pT", name="pT")
                    nc.vector.tensor_tensor(out=zsq[:, 0, :], in0=zT[dt_][:],
                                            in1=zT[dt_][:], op=OP.mult)
                    zsqs.append(zsq)
                sum_ps = pool2.tile([P, 512], F32, tag="t2", name="t2")
                for dt_ in range(DT):
                    nc.tensor.matmul(sum_ps[0:1, :], ones_col[:], zT[dt_][:],
                                     start=(dt_ == 0), stop=(dt_ == DT - 1))
                sq_ps = pool2.tile([P, 512], F32, tag="t2", name="t2")
                for dt_ in range(DT):
                    nc.tensor.matmul(sq_ps[0:1, :], ones_col[:],
                                     zsqs[dt_][:, 0, :],
                                     start=(dt_ == 0), stop=(dt_ == DT - 1))
                mean = ln1.tile([1, 512], F32, tag="mean", name="mean")
                nc.scalar.mul(mean[:], sum_ps[0:1, :], 1.0 / D)
                msq = ln1.tile([1, 512], F32, tag="msq", name="msq")
                nc.scalar.mul(msq[:], sq_ps[0:1, :], 1.0 / D)
                mean_r = ln1.tile([1, 512], BF16, tag="meanr", name="meanr")
                nc.vector.tensor_copy(mean_r[:], mean[:])
                # in-place: mean <- mean^2 ; msq <- var ; mean <- rstd
                nc.vector.tensor_mul(mean[:], mean[:], mean[:])
                nc.vector.tensor_sub(msq[:], msq[:], mean[:])
                std = ln1.tile([1, 512], F32, tag="std", name="std")
                nc.scalar.activation(std[:], msq[:], AF.Sqrt,
                                     bias=eps_sb[0:1, :])
                nc.vector.reciprocal(mean[:], std[:])
                rstd_r = ln1.tile([1, 512], BF16, tag="rstdr", name="rstdr")
                nc.vector.tensor_copy(rstd_r[:], mean[:])
                bm_ps = pool2.tile([P, 512], F32, tag="t2", name="t2")
                nc.tensor.matmul(bm_ps[:], ones_row[:], mean_r[:],
                                 start=True, stop=True)
                br_ps = pool2.tile([P, 512], F32, tag="t2", name="t2")
                nc.tensor.matmul(br_ps[:], ones_row[:], rstd_r[:],
                                 start=True, stop=True)
                bm = ln1.tile([P, 512], BF16, tag="bm_sb", name="bm_sb")
                nc.vector.tensor_copy(bm[:], bm_ps[:])
                br = ln1.tile([P, 512], BF16, tag="br_sb", name="br_sb")
                nc.vector.tensor_copy(br[:], br_ps[:])
                stats1.bm, stats1.br = bm, br
            units.append(stats1)

            def norm1(dt_):
                bm, br = stats1.bm, stats1.br
                tmp = ln1.tile([P, 512], BF16, tag="n1", name="n1", bufs=2)
                nc.vector.scalar_tensor_tensor(
                    tmp[:], zT[dt_][:], 1.0, bm[:],
                    op0=OP.mult, op1=OP.subtract)
                nc.vector.scalar_tensor_tensor(
                    hT[dt_][:, qsl], tmp[:],
                    biases["g1_p"][:, dt_:dt_ + 1], br[:],
                    op0=OP.mult, op1=OP.mult)
                nc.vector.tensor_scalar(
                    out=hT[dt_][:, qsl], in0=hT[dt_][:, qsl],
                    scalar1=biases["be1_p"][:, dt_:dt_ + 1], scalar2=None,
                    op0=OP.add)
            for d0 in range(0, DT, 2):
                units.append((lambda d=d0: (norm1(d), norm1(d + 1))))
            return units

        # =============== FFN machinery (pairs of 512-wide blocks) =======
        def ffn_units(qc, wx1_pool, wx2_pool, t1_pool):
            qsl = slice(qc * 512, (qc + 1) * 512)
            out2[qc] = [o2_pool.tile([P, 512], BF16, tag=f"o2_{qc}_{i}",
                                     name=f"o2_{qc}_{i}") for i in range(DT)]
            units = []
            for hbp in range(4):          # block pair: hidden 1024 = 8 tiles
                w1b = [wx1_pool.tile([P, 512], BF16, tag="wx1", name="wx1")
                       for _ in range(2 * DT)]
                w2b = [wx2_pool.tile([P, D], BF16, tag="wx2", name="wx2")
                       for _ in range(DT)]
                t1s = []

                def dma_w(half, hbp=hbp, w1b=w1b, w2b=w2b):
                    hb = 2 * hbp + half
                    for i in range(DT):
                        nc.gpsimd.dma_start(
                            out=w1b[half * DT + i],
                            in_=w1[i * P:(i + 1) * P,
                                   hb * 512:(hb + 1) * 512])
                    for i in range(4):
                        nc.gpsimd.dma_start(
                            out=w2b[half * 4 + i],
                            in_=w2[(hb * 4 + i) * P:(hb * 4 + i + 1) * P, :])

                def t1_group(half, i, hbp=hbp, w1b=w1b, t1s=t1s):
                    hb = 2 * hbp + half
                    ps = pool2.tile([P, 512], F32, tag="t2", name="t2")
                    for din in range(DT):
                        nc.tensor.matmul(
                            ps[:], w1b[half * DT + din][:, i * P:(i + 1) * P],
                            hT[din][:, qsl],
                            start=(din == 0), stop=(din == DT - 1))
                    t1 = t1_pool.tile([P, 512], BF16, tag="t1", name="t1")
                    if qc == 1:
                        # scalar engine is idle after the softmaxes
                        nc.scalar.activation(
                            t1[:], ps[:], AF.Relu,
                            bias=b1_sb[:, hb * 4 + i:hb * 4 + i + 1])
                    else:
                        nc.vector.tensor_scalar(
                            out=t1[:], in0=ps[:],
                            scalar1=b1_sb[:, hb * 4 + i:hb * 4 + i + 1],
                            scalar2=0.0, op0=OP.add, op1=OP.max)
                    t1s.append(t1)

                def o2_group(dout, hbp=hbp, w2b=w2b, t1s=t1s):
                    ps = pool2.tile([P, 512], F32, tag="t2", name="t2")
                    for i in range(DT):
                        nc.tensor.matmul(
                            ps[:], w2b[i][:, dout * P:(dout + 1) * P],
                            t1s[i][:], start=(i == 0), stop=(i == DT - 1))
                    od = out2[qc][dout]
                    if hbp == 0:
                        nc.vector.tensor_copy(od[:], ps[:])
                    elif hbp < 3:
                        nc.vector.tensor_tensor(out=od[:], in0=ps[:],
                                                in1=od[:], op=OP.add)
                    else:
                        # last pair: fold in  + b2 + h  (residual)
                        nc.vector.scalar_tensor_tensor(
                            od[:], ps[:], biases["b2_p"][:, dout:dout + 1],
                            od[:], op0=OP.add, op1=OP.add)
                        nc.vector.tensor_tensor(out=od[:], in0=od[:],
                                                in1=hT[dout][:, qsl],
                                                op=OP.add)
                for half in range(2):
                    units.append((lambda h=half, f=dma_w: f(h)))
                    for i in range(4):
                        units.append((lambda h=half, i=i, f=t1_group:
                                      f(h, i)))
                for dout in range(DT):
                    units.append((lambda d=dout, f=o2_group: f(d)))
            return units

        # ====== P4: attention chunk 1 + [Wo/LN1 chunk 0, FFN chunk 0] ===
        zt0_cm = tc.tile_pool(name="zT0", bufs=1)
        zt0_pool = zt0_cm.__enter__()
        wxa1_cm = tc.tile_pool(name="wxa1", bufs=10)
        wxa1 = wxa1_cm.__enter__()
        wxa2_cm = tc.tile_pool(name="wxa2", bufs=8)
        wxa2 = wxa2_cm.__enter__()
        t1a_cm = tc.tile_pool(name="t1a", bufs=10)
        t1a = t1a_cm.__enter__()

        work0 = deque(wo_ln1_units(0, dens0, zt0_pool))
        work0.extend(ffn_units(0, wxa1, wxa2, t1a))
        ksb_cur = load_ksb(0)
        dens1 = {}
        for p in range(DT):
            fillers = deque()
            for _ in range(12):
                if work0:
                    fillers.append(work0.popleft())
            if p - 1 in dens1:
                fillers.append(
                    (lambda d=p - 1: normalize(d, 1, dens1.pop(d))))
            dens1[p] = attn_pair(p, 1, ksb_cur, fillers)
            while fillers:
                fillers.popleft()()
            if p < DT - 1:
                ksb_cur = load_ksb(p + 1)
        while work0:
            work0.popleft()()

        t1a_cm.__exit__(None, None, None)
        wxa2_cm.__exit__(None, None, None)
        wxa1_cm.__exit__(None, None, None)
        zt0_cm.__exit__(None, None, None)

        # =============== P5: Wo + LN1 (chunk 1) =========================
        zt1_cm = tc.tile_pool(name="zT1", bufs=1)
        for u in wo_ln1_units(1, dens1, zt1_cm.__enter__()):
            u()
        zt1_cm.__exit__(None, None, None)
        wo_cm.__exit__(None, None, None)
        ln1_cm.__exit__(None, None, None)
        pool3_cm.__exit__(None, None, None)
        pool1_cm.__exit__(None, None, None)
        qt1_cm.__exit__(None, None, None)
        v_cm.__exit__(None, None, None)
        kb_cm.__exit__(None, None, None)
        ctx_cm.__exit__(None, None, None)

        # =============== tail pools =====================================
        pool6 = es.enter_context(
            tc.tile_pool(name="pool6", bufs=4, space="PSUM"))
        ln2_pool = es.enter_context(tc.tile_pool(name="ln2", bufs=1))
        z2m_pool = es.enter_context(tc.tile_pool(name="z2m", bufs=2))
        wxb1_cm = tc.tile_pool(name="wxb1", bufs=12)
        wxb1 = wxb1_cm.__enter__()
        wxb2_cm = tc.tile_pool(name="wxb2", bufs=10)
        wxb2 = wxb2_cm.__enter__()
        t1b_cm = tc.tile_pool(name="t1b", bufs=10)
        t1b = t1b_cm.__enter__()

        def ln2_units(qc):
            units = []

            def ln2_tile(nt):
                z2 = z2m_pool.tile([P, D], F32, tag="z2tm", name="z2tm")
                for dt_ in range(DT):
                    pt = pool6.tile([P, P], BF16, tag="psT5", name="psT5")
                    nc.tensor.transpose(
                        pt[:], out2[qc][dt_][:, nt * P:(nt + 1) * P],
                        ident[:])
                    nc.scalar.copy(z2[:, dt_ * P:(dt_ + 1) * P], pt[:])
                stats = ln2_pool.tile([P, 2, 6], F32, tag="st", name="st")
                for g in range(2):
                    nc.vector.bn_stats(out=stats[:, g, :],
                                       in_=z2[:, g * 512:(g + 1) * 512])
                mv = ln2_pool.tile([P, 2], F32, tag="mv", name="mv")
                nc.vector.bn_aggr(out=mv[:], in_=stats[:])
                std = ln2_pool.tile([P, 1], F32, tag="std2", name="std2")
                nc.scalar.activation(std[:], mv[:, 1:2], AF.Sqrt,
                                     bias=eps_sb[:])
                rstd = ln2_pool.tile([P, 1], F32, tag="rstd2", name="rstd2")
                nc.vector.reciprocal(rstd[:], std[:])
                xn = ln2_pool.tile([P, D], F32, tag="xn", name="xn", bufs=2)
                nc.vector.tensor_scalar(
                    out=xn[:], in0=z2[:], scalar1=mv[:, 0:1],
                    scalar2=rstd[:], op0=OP.subtract, op1=OP.mult)
                nc.vector.tensor_mul(xn[:], xn[:], g2_bc[:])
                nc.vector.tensor_add(xn[:], xn[:], be2_bc[:])
                nt_g = qc * 4 + nt
                nc.sync.dma_start(out=out[nt_g * P:(nt_g + 1) * P, :],
                                  in_=xn[:])
            for nt in range(4):
                units.append((lambda n=nt: ln2_tile(n)))
            return units

        # =============== P5b: FFN chunk 1 + LN2 chunk 0 =================
        ln2_0 = deque(ln2_units(0))
        fu1 = ffn_units(1, wxb1, wxb2, t1b)
        for ui, u in enumerate(fu1):
            u()
            if ln2_0 and ui % 9 == 8:
                ln2_0.popleft()()
        while ln2_0:
            ln2_0.popleft()()

        # =============== P6: LN2 chunk 1 (tail) =========================
        for u in ln2_units(1):
            u()

        t1b_cm.__exit__(None, None, None)
        wxb2_cm.__exit__(None, None, None)
        wxb1_cm.__exit__(None, None, None)


def _pack(v, nt):
    return np.ascontiguousarray(v.reshape(nt, P).T)


def _sel2():
    s = np.zeros((2, P), dtype=np.float32)
    s[0, 0:HD] = 1.0
    s[1, HD:P] = 1.0
    return s


def kernel(x, Wq, bq, Wk, bk, Wv, bv, Wo, bo, W1, b1, W2, b2, g1, beta1,
           g2, beta2):
    x = np.asarray(x, dtype=np.float32)
    if "nc" not in _CACHED:
        _CACHED["nc"] = _build_program()
    nc = _CACHED["nc"]

    import ml_dtypes
    bf16 = lambda a: np.ascontiguousarray(
        np.asarray(a, dtype=np.float32).astype(ml_dtypes.bfloat16))
    f32 = lambda a: np.ascontiguousarray(np.asarray(a, dtype=np.float32))
    scale = 1.0 / np.sqrt(HD)
    common = {
        "wq": bf16(Wq), "wk": bf16(np.asarray(Wk, np.float64) * scale),
        "wv": bf16(Wv), "wo": bf16(Wo), "w1": bf16(W1), "w2": bf16(W2),
        "bq_p": _pack(f32(bq), DT), "bk_p": _pack(f32(bk) * scale, DT),
        "bo_p": _pack(f32(bo), DT),
        "b1_p": _pack(f32(b1), FT), "b2_p": _pack(f32(b2), DT),
        "g1_p": _pack(f32(g1), DT), "be1_p": _pack(f32(beta1), DT),
        "bv_r": bf16(np.asarray(bv).reshape(1, D)),
        "g2_d": bf16(g2), "be2_d": bf16(beta2),
        "ident_d": np.eye(P).astype(ml_dtypes.bfloat16),
        "ones_row_d": np.ones((1, P)).astype(ml_dtypes.bfloat16),
        "ones_col_d": np.ones((P, 1)).astype(ml_dtypes.bfloat16),
        "sel2_d": _sel2().astype(ml_dtypes.bfloat16),
    }
    in_maps = []
    for c in range(N_CORES):
        b, half = c // 2, c % 2
        own = x[b, half * T:(half + 1) * T]           # [1024, 1024]
        other = x[b, (1 - half) * T:(2 - half) * T]
        xT_c = np.ascontiguousarray(
            np.concatenate([own, other], axis=0).T).astype(
                ml_dtypes.bfloat16)                   # [1024, 2048]
        in_maps.append({**common, "xT": xT_c})

    trace = bool(os.environ.get("KERNEL_TRACE"))
    res = bass_utils.run_bass_kernel_spmd(
        nc, in_maps, core_ids=list(range(N_CORES)), trace=trace)
    _CACHED["last_result"] = res

    y = np.empty((4, S, D), dtype=np.float32)
    for c in range(N_CORES):
        b, half = c // 2, c % 2
        y[b, half * T:(half + 1) * T] = res.results[c]["out"]
    return y
